# revision 11
# baseline (speedup 1.0000x reference)
"""SpaceFillingVQ Trainium2 kernel (8-core data-parallel over rows).

Strategy: shard input_data rows 8 ways; replicate dithered-codebook build on
each core. Per core: f32r score matmul (x @ dcb.T - c2/2) on the PE into
PSUM, fp16 egress on the scalar engine, top-8 coarse argmax (DVE max/
max_index), exact fp32 rescore of the top-4 candidates via indirect-DMA
gather of [dcb | -c2/2] table rows, decode gather, and a factorized one-hot
matmul histogram. Host sums per-core histograms and computes perplexity.
"""
import os
import sys
import numpy as np

for _p in ("/opt/trn_rl_repo", "/root/.axon_site/_ro/trn_rl_repo"):
    if os.path.isdir(_p) and _p not in sys.path:
        sys.path.insert(0, _p)

import concourse.bass as bass
import concourse.mybir as mybir
from concourse import bacc
from concourse.bass import ts, ds
from concourse.tile import TileContext
from concourse.masks import make_identity

f32 = mybir.dt.float32
f32r = mybir.dt.float32r
f16 = mybir.dt.float16
bf16 = mybir.dt.bfloat16
u16 = mybir.dt.uint16
i32 = mybir.dt.int32

N_CORES = 8
N_ROWS = 65536
D = 64
E = 4096
R = N_ROWS // N_CORES  # rows per core


def _build(nc, R):
    T = R // 128
    xT_d = nc.dram_tensor("xT", [65, R], f32, kind="ExternalInput")
    xrow_d = nc.dram_tensor("xrow", [R, 64], f32, kind="ExternalInput")
    cbT_d = nc.dram_tensor("cbT", [64, E], f32, kind="ExternalInput")
    dr_d = nc.dram_tensor("dr", [64, E], f32, kind="ExternalInput")
    dr1_d = nc.dram_tensor("dr1", [64, E], f32, kind="ExternalInput")
    quant_d = nc.dram_tensor("quant", [R, 64], f32, kind="ExternalOutput")
    idx_d = nc.dram_tensor("idx", [R], i32, kind="ExternalOutput")
    counts_d = nc.dram_tensor("counts", [64, 64], f32, kind="ExternalOutput")
    tbl_d = nc.dram_tensor("tbl", [E, 64], f32, kind="Internal")
    scr_d = nc.dram_tensor("scr", [64 * 128], mybir.dt.int16, kind="Internal")

    with TileContext(nc) as tc:
        with tc.tile_pool(name="keep", bufs=1) as keep:
            EIDX8 = keep.tile([128, T * 8], u16)
            EB = keep.tile([128, T], f32)
            IDENT = keep.tile([128, 128], f32)
            make_identity(nc, IDENT[:])
            IOTA = keep.tile([128, 64], i32)
            nc.gpsimd.iota(IOTA[:], pattern=[[1, 64]], base=0, channel_multiplier=0)
            IOTAF = keep.tile([128, 1, 64], f32)
            nc.vector.tensor_copy(IOTAF[:, 0, :], IOTA[:])

            with tc.tile_pool(name="mm", bufs=1) as mm:
                with (
                    tc.tile_pool(name="prep", bufs=1) as prep,
                    tc.tile_pool(name="prep2", bufs=2) as prep2,
                    tc.tile_pool(name="pp", bufs=2, space="PSUM") as pp,
                ):
                    XAR = mm.tile([65, R], f32r)
                    AUGF = mm.tile([65, E], f32)
                    AUGR = mm.tile([65, E], f32r)

                    XA = prep.tile([65, R], f32)
                    nc.sync.dma_start(XA[:], xT_d[:])
                    nc.vector.tensor_copy(XAR[:], XA[:])
                    CT = prep.tile([64, E], f32)
                    DR = prep.tile([64, E], f32)
                    DR1 = prep.tile([64, E], f32)
                    nc.sync.dma_start(CT[:], cbT_d[:])
                    nc.sync.dma_start(DR[:], dr_d[:])
                    nc.sync.dma_start(DR1[:], dr1_d[:])
                    t1 = prep.tile([64, E], f32)
                    nc.vector.tensor_mul(t1[:, 0:E - 1], DR[:, 0:E - 1], CT[:, 1:E])
                    t2 = prep.tile([64, E], f32)
                    nc.vector.tensor_mul(t2[:, 0:E - 1], DR1[:, 0:E - 1], CT[:, 0:E - 1])
                    nc.vector.tensor_add(AUGF[0:64, 0:E - 1], t1[:, 0:E - 1], t2[:, 0:E - 1])
                    nc.vector.memset(AUGF[0:64, E - 1:E], 0.0)
                    SQ = prep.tile([64, E], f32)
                    nc.vector.tensor_mul(SQ[:], AUGF[0:64, :], AUGF[0:64, :])
                    ONES = prep.tile([64, 1], f32)
                    nc.vector.memset(ONES[:], 1.0)
                    for b in range(E // 512):
                        c2p = pp.tile([1, 512], f32, tag="c2")
                        nc.tensor.matmul(c2p[:], ONES[:], SQ[:, ts(b, 512)],
                                         start=True, stop=True)
                        nc.scalar.mul(AUGF[64:65, ts(b, 512)], c2p[:], -0.5)
                    nc.vector.memset(AUGF[64:65, E - 1:E], -60000.0)
                    nc.vector.tensor_copy(AUGR[:], AUGF[:])

                    for ch in range(E // 128):
                        tp = pp.tile([128, 64], f32, tag="tp")
                        nc.tensor.transpose(tp[:], AUGF[0:64, ts(ch, 128)], IDENT[0:64, 0:64])
                        stg = prep2.tile([128, 64], f32, tag="stg")
                        nc.vector.tensor_copy(stg[:], tp[:])
                        nc.sync.dma_start(tbl_d[ts(ch, 128), :], stg[:])

                with (
                    tc.tile_pool(name="lp", bufs=3) as lp,
                    tc.tile_pool(name="psA", bufs=2, space="PSUM") as psA,
                ):
                    for t in range(T):
                        F = lp.tile([128, E], f32, tag="F")
                        for h in range(2):
                            ps = psA.tile([128, 2048], f32, tag="ps")
                            for b in range(4):
                                nc.tensor.matmul(
                                    ps[:, ts(b, 512)],
                                    XAR[:, ts(t, 128)],
                                    AUGR[:, ds(h * 2048 + b * 512, 512)],
                                    start=True, stop=True)
                            nc.scalar.copy(F[:, ds(h * 2048, 2048)], ps[:])
                        MX = lp.tile([128, 8], f32, tag="MX")
                        nc.vector.max(out=MX[:], in_=F[:])
                        nc.vector.max_index(EIDX8[:, ts(t, 8)], MX[:], F[:])

            NG = max(1, T // 16)
            TG = T // NG
            i16 = mybir.dt.int16
            with (
                tc.tile_pool(name="p2", bufs=2) as p2,
                tc.tile_pool(name="p2b", bufs=1) as p2b,
                tc.tile_pool(name="pq", bufs=2, space="PSUM") as pq,
            ):
                def bulk_gather(nc, offs_f32, J, out_tile):
                    """out_tile[p, j, :] = TBL[offs_f32[p, j]] for J<=128 cols."""
                    tj = pq.tile([J, 128], f32, tag="tj")
                    nc.tensor.transpose(tj[:], offs_f32, IDENT[:])
                    tj16 = p2.tile([J, 128], i16, tag="tj16")
                    nc.vector.tensor_copy(tj16[:], tj[:])
                    nc.sync.dma_start(scr_d[0:J * 128].rearrange("(j p) -> j p", p=128), tj16[:])
                    idxs = p2.tile([128, J * 8], i16, tag="idxs")
                    for gq in range(8):
                        nc.sync.dma_start(
                            idxs[ds(gq * 16, 16), :],
                            scr_d[0:J * 128].rearrange("(s q) -> q s", q=16))
                    nc.gpsimd.dma_gather(
                        out_ap=out_tile, in_ap=tbl_d[:], idxs_ap=idxs[:],
                        num_idxs=J * 128, num_idxs_reg=J * 128, elem_size=64,
                        single_packet=False)

                for g in range(NG):
                    eview = EIDX8[:, ds(g * TG * 8, TG * 8)].rearrange(
                        "p (t c) -> p t c", c=8)[:, :, 0:4]
                    EF = p2.tile([128, TG, 4], f32, tag="EF")
                    nc.vector.tensor_copy(EF[:], eview)
                    GR = p2.tile([128, TG * 4, 64], f32, tag="GR")
                    bulk_gather(nc, EF[:].rearrange("p t c -> p (t c)"), TG * 4, GR[:])
                    XRg = p2.tile([128, TG, 1, 64], f32, tag="XRg")
                    nc.sync.dma_start(
                        XRg[:, :, 0, :],
                        xrow_d[ds(g * TG * 128, TG * 128), :].rearrange(
                            "(t p) d -> p t d", p=128))
                    # dots = sum_d GR*x ; c2c = sum_d GR*GR
                    P4 = p2.tile([128, TG, 4, 64], f32, tag="P4")
                    nc.vector.tensor_mul(
                        P4[:], GR[:].rearrange("p (t c) d -> p t c d", c=4),
                        XRg[:].to_broadcast([128, TG, 4, 64]))
                    DOTS = p2.tile([128, TG, 4], f32, tag="DOTS")
                    nc.vector.tensor_reduce(
                        DOTS[:], P4[:], axis=mybir.AxisListType.X,
                        op=mybir.AluOpType.add)
                    P4b = p2.tile([128, TG, 4, 64], f32, tag="P4b")
                    nc.vector.tensor_mul(
                        P4b[:], GR[:].rearrange("p (t c) d -> p t c d", c=4),
                        GR[:].rearrange("p (t c) d -> p t c d", c=4))
                    C2C = p2.tile([128, TG, 4], f32, tag="C2C")
                    nc.vector.tensor_reduce(
                        C2C[:], P4b[:], axis=mybir.AxisListType.X,
                        op=mybir.AluOpType.add)
                    R4 = p2.tile([128, TG, 4], f32, tag="R4")
                    nc.vector.scalar_tensor_tensor(
                        out=R4[:], in0=C2C[:], scalar=-0.5, in1=DOTS[:],
                        op0=mybir.AluOpType.mult, op1=mybir.AluOpType.add)
                    MM = p2.tile([128, TG, 1], f32, tag="MM")
                    nc.vector.tensor_reduce(
                        MM[:], R4[:], axis=mybir.AxisListType.X,
                        op=mybir.AluOpType.max)
                    EQ = p2.tile([128, TG, 4], f32, tag="EQ")
                    nc.vector.tensor_tensor(
                        out=EQ[:], in0=R4[:],
                        in1=MM[:].to_broadcast([128, TG, 4]),
                        op=mybir.AluOpType.is_ge)
                    V = p2.tile([128, TG, 4], f32, tag="V")
                    nc.vector.tensor_scalar_add(V[:], EF[:], 8192.0)
                    MK = p2.tile([128, TG, 4], f32, tag="MK")
                    nc.vector.scalar_tensor_tensor(
                        out=MK[:], in0=EQ[:], scalar=-8192.0, in1=V[:],
                        op0=mybir.AluOpType.mult, op1=mybir.AluOpType.add)
                    nc.vector.tensor_reduce(
                        EB[:, ds(g * TG, TG)], MK[:], axis=mybir.AxisListType.X,
                        op=mybir.AluOpType.min)

                Q = p2b.tile([128, T, 64], f32)
                bulk_gather(nc, EB[:], T, Q[:])
                nc.sync.dma_start(
                    quant_d[:].rearrange("(t p) d -> p t d", p=128), Q[:])
                tpe = pq.tile([T, 128], f32, tag="tpe")
                nc.tensor.transpose(tpe[:], EB[:], IDENT[:])
                I32 = p2b.tile([T, 128], i32)
                nc.vector.tensor_copy(I32[:], tpe[:])
                nc.sync.dma_start(
                    idx_d[:].rearrange("(t p) -> t p", p=128), I32[:])
                # h = floor(EB/64), robust to either int-convert rounding mode
                T1 = p2b.tile([128, T, 1], f32)
                nc.vector.tensor_scalar_mul(
                    T1[:], EB[:].rearrange("p (t o) -> p t o", o=1), 1.0 / 64.0)
                Hi = p2b.tile([128, T, 1], i32)
                nc.vector.tensor_copy(Hi[:], T1[:])
                Hh = p2b.tile([128, T, 1], f32)
                nc.vector.tensor_copy(Hh[:], Hi[:])
                DGT = p2b.tile([128, T, 1], f32)
                nc.vector.tensor_tensor(out=DGT[:], in0=Hh[:], in1=T1[:],
                                        op=mybir.AluOpType.is_gt)
                nc.vector.tensor_sub(Hh[:], Hh[:], DGT[:])
                L = p2b.tile([128, T, 1], f32)
                nc.vector.scalar_tensor_tensor(
                    out=L[:], in0=Hh[:], scalar=-64.0,
                    in1=EB[:].rearrange("p (t o) -> p t o", o=1),
                    op0=mybir.AluOpType.mult, op1=mybir.AluOpType.add)
                IH = p2b.tile([128, T, 64], bf16)
                nc.vector.tensor_tensor(
                    out=IH[:], in0=Hh[:].to_broadcast([128, T, 64]),
                    in1=IOTAF[:].to_broadcast([128, T, 64]),
                    op=mybir.AluOpType.is_equal)
                IL = p2b.tile([128, T, 64], bf16)
                nc.vector.tensor_tensor(
                    out=IL[:], in0=L[:].to_broadcast([128, T, 64]),
                    in1=IOTAF[:].to_broadcast([128, T, 64]),
                    op=mybir.AluOpType.is_equal)
                cps = pq.tile([64, 64], f32, tag="cnt")
                for t in range(T):
                    nc.tensor.matmul(cps[:], IH[:, t, :], IL[:, t, :],
                                     start=(t == 0), stop=(t == T - 1))
                CS = p2b.tile([64, 64], f32)
                nc.vector.tensor_copy(CS[:], cps[:])
                nc.sync.dma_start(counts_d[:], CS[:])
    return nc


_CACHE = {}


def _get_runner():
    """Build the Bass program once and cache a jitted 8-core runner."""
    if "run" in _CACHE:
        return _CACHE["run"]
    nc = bacc.Bacc("TRN2", target_bir_lowering=False, debug=False,
                   num_devices=N_CORES)
    _build(nc, R)
    nc.compile()

    import jax
    from jax.sharding import Mesh, PartitionSpec
    from jax.experimental.shard_map import shard_map
    from concourse import bass2jax
    from concourse.bass2jax import _bass_exec_p, partition_id_tensor

    bass2jax.install_neuronx_cc_hook()

    partition_name = (nc.partition_id_tensor.name
                      if nc.partition_id_tensor else None)
    in_names, out_names, out_avals, zero_outs = [], [], [], []
    for alloc in nc.m.functions[0].allocations:
        if not isinstance(alloc, mybir.MemoryLocationSet):
            continue
        name = alloc.memorylocations[0].name
        if alloc.kind == "ExternalInput":
            if name != partition_name:
                in_names.append(name)
        elif alloc.kind == "ExternalOutput":
            shape = tuple(alloc.tensor_shape)
            dtype = mybir.dt.np(alloc.dtype)
            out_names.append(name)
            out_avals.append(jax.core.ShapedArray(shape, dtype))
            zero_outs.append(np.zeros(shape, dtype))
    n_params = len(in_names)
    n_outs = len(out_avals)
    all_in_names = list(in_names) + list(out_names)
    if partition_name is not None:
        all_in_names.append(partition_name)

    def _body(*args):
        operands = list(args)
        if partition_name is not None:
            operands.append(partition_id_tensor())
        outs = _bass_exec_p.bind(
            *operands,
            out_avals=tuple(out_avals),
            in_names=tuple(all_in_names),
            out_names=tuple(out_names),
            lowering_input_output_aliases=(),
            sim_require_finite=False,
            sim_require_nnan=False,
            nc=nc,
        )
        return tuple(outs)

    devices = jax.devices()[:N_CORES]
    mesh = Mesh(np.asarray(devices), ("core",))
    donate = tuple(range(n_params, n_params + n_outs))
    sharded = jax.jit(
        shard_map(_body, mesh=mesh,
                  in_specs=(PartitionSpec("core"),) * (n_params + n_outs),
                  out_specs=(PartitionSpec("core"),) * n_outs,
                  check_rep=False),
        donate_argnums=donate, keep_unused=True)

    def run(in_maps):
        concat_in = [
            np.concatenate([np.asarray(in_maps[c][nm]) for c in range(N_CORES)],
                           axis=0)
            for nm in in_names
        ]
        concat_zeros = [
            np.zeros((N_CORES * z.shape[0], *z.shape[1:]), z.dtype)
            for z in zero_outs
        ]
        out_arrs = sharded(*concat_in, *concat_zeros)
        return [
            {nm: np.asarray(out_arrs[i]).reshape(N_CORES, *out_avals[i].shape)[c]
             for i, nm in enumerate(out_names)}
            for c in range(N_CORES)
        ]

    _CACHE["run"] = run
    return run


def _host_inputs(x_shard, cbT, dr, dr1):
    Rr = x_shard.shape[0]
    xT = np.concatenate([x_shard.T, np.ones((1, Rr), np.float32)], axis=0)
    xT = np.ascontiguousarray(xT)
    return {"xT": xT, "xrow": np.ascontiguousarray(x_shard), "cbT": cbT,
            "dr": dr, "dr1": dr1}


def kernel(input_data, codebook, dither, entries):
    x = np.asarray(input_data, dtype=np.float32)
    cb = np.asarray(codebook, dtype=np.float32)
    dith = np.asarray(dither, dtype=np.float32)
    assert x.shape == (N_ROWS, D) and cb.shape == (E, D)
    assert int(entries) == E

    cbT = np.ascontiguousarray(cb.T)
    r = np.zeros((E,), np.float32)
    r[:E - 1] = dith
    dr = np.ascontiguousarray(np.broadcast_to(r, (64, E)))
    one_minus = (np.float32(1.0) - r).astype(np.float32)
    one_minus[E - 1] = 0.0
    dr1 = np.ascontiguousarray(np.broadcast_to(one_minus, (64, E)))

    in_maps = [
        _host_inputs(np.ascontiguousarray(x[c * R:(c + 1) * R]), cbT, dr, dr1)
        for c in range(N_CORES)
    ]
    run = _get_runner()
    results = run(in_maps)

    quantized = np.concatenate([results[c]["quant"] for c in range(N_CORES)],
                               axis=0)
    integer_index = np.concatenate([results[c]["idx"] for c in range(N_CORES)],
                                   axis=0).astype(np.int32)
    counts = np.zeros((E,), np.float32)
    for c in range(N_CORES):
        counts += results[c]["counts"].reshape(-1)
    avg_probs = (counts / np.float32(N_ROWS)).astype(np.float32)
    perplexity = np.float32(np.exp(-np.sum(
        avg_probs * np.log(avg_probs + np.float32(1e-10)), dtype=np.float32)))
    return quantized, perplexity, integer_index
